# revision 1
# baseline (speedup 1.0000x reference)
"""MoE grouped-experts kernel for Trainium2 (8 NeuronCores, expert-parallel).

Strategy
--------
Expert-parallel: 32 experts packed onto 8 cores x 4 slots. Routing
(sort-by-expert, capacity truncation at the reference's C=1024) is computed
on host from the tiny `indices` tensor; token rows are gathered per expert,
padded to the slot capacity, and pre-transposed so the device kernel is a
pure stream of fp32r matmuls with zero on-device transposes:

  GEMM1 (h^T orientation):  hT[m,c] = sum_k gup[k,m] * xT[k,c]
        stationary = gup tile [128 D, 128 cols-of-2I], moving = xT tokens
  act:  aT = silu(1.702*min(gate,7)) * (clip(up,-7,7)+1)   (the 1/1.702 is
        folded into the routing probs applied at GEMM2 eviction)
  GEMM2: y[c,d] = sum_k aT[k,c] * down[k,d], eviction scaled by probs/1.702.

Slot capacities adapt to the actual expert loads (same structure on every
core - SPMD): slot j's capacity = max token-block count among the experts
assigned to slot j across cores. All matmuls run as float32r (single-pass,
full PE rate at moving dim >= 256) with fp32 PSUM accumulation; accumulation
groups are interleaved in pairs so the 4-byte weight loads hide under the
previous matmul's streaming.
"""

import math
from contextlib import ExitStack

import numpy as np

N_TOKENS, DIM = 4096, 2048
N_EXPERTS, TOPK, INTER = 32, 4, 1408
ALPHA, LIMIT, LIN_OFFSET = 1.702, 7.0, 1.0

NCORE = 8
NSLOT = N_EXPERTS // NCORE        # expert slots per core = 4
KD = DIM // 128                   # 16 contraction tiles for GEMM1
KI = INTER // 128                 # 11 contraction tiles for GEMM2
DW = 512                          # GEMM2 moving-dim chunk over DIM
NDC = DIM // DW                   # 4
C_REF = 2 * ((N_TOKENS * TOPK + N_EXPERTS - 1) // N_EXPERTS)  # 1024

_PROG_CACHE: dict = {}


def _token_groups(cap: int):
    """Split cap into moving-dim groups, each <= 512 and >= 256 (fp32r full rate)."""
    ng = max(1, math.ceil(cap / 512))
    base = cap // ng
    sizes = [base] * ng
    for i in range(cap - base * ng):
        sizes[i] += 1
    out, off = [], 0
    for s in sizes:
        out.append((off, s))
        off += s
    return out


def _build_program(caps: tuple):
    import concourse.bacc as bacc
    import concourse.mybir as mybir
    import concourse.tile as tile
    from concourse.alu_op_type import AluOpType

    F32 = mybir.dt.float32
    F32R = mybir.dt.float32r
    SB = sum(caps)                      # total 128-row blocks per core
    cmax = max(caps) * 128
    xt_sizes = [128 * KD * c * 128 for c in caps]
    xt_off = np.concatenate([[0], np.cumsum(xt_sizes)]).tolist()
    soff = np.concatenate([[0], np.cumsum(caps)]).tolist()  # block offsets

    nc = bacc.Bacc(None, target_bir_lowering=False, debug=False)
    with ExitStack() as ctx:
        tc = ctx.enter_context(tile.TileContext(nc))
        dram = ctx.enter_context(tc.tile_pool(name="dram", bufs=1, space="DRAM"))
        xt_d = dram.tile([xt_off[-1]], F32R, kind="ExternalInput")
        gup_d = dram.tile([NSLOT, 2, KI, 128, KD * 128], F32R, kind="ExternalInput")
        down_d = dram.tile([NSLOT, NDC, 128, KI * DW], F32R, kind="ExternalInput")
        probs_d = dram.tile([128, SB], F32, kind="ExternalInput")
        y_d = dram.tile([SB, 128, DIM], F32, kind="ExternalOutput")
        names = {
            "xt": xt_d.name, "gup": gup_d.name, "down": down_d.name,
            "probs": probs_d.name, "y": y_d.name,
        }

        xt_pool = ctx.enter_context(tc.tile_pool(name="xt", bufs=3))
        gup_pool = ctx.enter_context(tc.tile_pool(name="gup", bufs=5))
        down_pool = ctx.enter_context(tc.tile_pool(name="down", bufs=2))
        at_pool = ctx.enter_context(tc.tile_pool(name="at", bufs=1))
        fg_pool = ctx.enter_context(tc.tile_pool(name="fg", bufs=3))
        tmp_pool = ctx.enter_context(tc.tile_pool(name="tmp", bufs=4))
        y_pool = ctx.enter_context(tc.tile_pool(name="yt", bufs=3))
        pr_pool = ctx.enter_context(tc.tile_pool(name="pr", bufs=1))
        psg1 = ctx.enter_context(tc.tile_pool(name="psg1", bufs=4, space="PSUM"))
        psg2 = ctx.enter_context(tc.tile_pool(name="psg2", bufs=3, space="PSUM"))

        probs_sb = pr_pool.tile([128, SB], F32)
        nc.sync.dma_start(out=probs_sb[:], in_=probs_d[:])

        for j in range(NSLOT):
            CAP = caps[j] * 128
            capb = caps[j]
            groups = _token_groups(CAP)
            half_elems = 128 * (KD // 2) * CAP

            # xT in two half-slabs (k 0..7 / k 8..15) for cheap cross-expert prefetch
            xt_h = []
            for h in (0, 1):
                t = xt_pool.tile([128, (KD // 2) * cmax], F32R, tag="xt")
                src = xt_d[xt_off[j] + h * half_elems: xt_off[j] + (h + 1) * half_elems]
                nc.sync.dma_start(
                    out=t[:, :(KD // 2) * CAP],
                    in_=src.rearrange("(p c) -> p c", p=128),
                )
                xt_h.append(t)

            def xt_ap(k, g0, gw, CAP=CAP, xt_h=xt_h):
                t = xt_h[k // (KD // 2)]
                kk = k % (KD // 2)
                return t[:, kk * CAP + g0: kk * CAP + g0 + gw]

            at_sb = at_pool.tile([128, KI * cmax], F32R, tag="at")

            for i in range(KI):
                for half in (0, 1):  # 0 = gate, 1 = up
                    gsb = gup_pool.tile([128, KD * 128], F32R, tag="gup")
                    nc.sync.dma_start(out=gsb[:], in_=gup_d[j, half, i])
                    pss = [psg1.tile([128, 512], F32, tag="ps1", name=f"ps1_{i}_{half}_{gi}") for gi in range(len(groups))]
                    for k in range(KD):
                        for gi, (g0, gw) in enumerate(groups):
                            nc.tensor.matmul(
                                pss[gi][:, :gw],
                                lhsT=gsb[:, k * 128:(k + 1) * 128],
                                rhs=xt_ap(k, g0, gw),
                                start=(k == 0), stop=(k == KD - 1),
                            )
                    for gi, (g0, gw) in enumerate(groups):
                        ps = pss[gi]
                        if half == 0:
                            t0 = tmp_pool.tile([128, 512], F32, tag="t0")
                            nc.vector.tensor_scalar_min(t0[:, :gw], ps[:, :gw], LIMIT)
                            fg = fg_pool.tile([128, 512], F32, tag="fg")
                            nc.scalar.activation(
                                fg[:, :gw], t0[:, :gw],
                                mybir.ActivationFunctionType.Silu, scale=ALPHA,
                            )
                            if gi == 0:
                                fgs = [fg]
                            else:
                                fgs.append(fg)
                        else:
                            uc = tmp_pool.tile([128, 512], F32, tag="uc")
                            nc.vector.tensor_scalar(
                                uc[:, :gw], ps[:, :gw], LIMIT, -LIMIT,
                                AluOpType.min, AluOpType.max,
                            )
                            # aT = (clip(up)+1) * silu(1.702*min(gate,7))
                            nc.vector.scalar_tensor_tensor(
                                at_sb[:, i * CAP + g0: i * CAP + g0 + gw],
                                uc[:, :gw], LIN_OFFSET, fgs[gi][:, :gw],
                                AluOpType.add, AluOpType.mult,
                            )

            for dc in range(NDC):
                dsb = down_pool.tile([128, KI * DW], F32R, tag="down")
                nc.sync.dma_start(out=dsb[:], in_=down_d[j, dc])
                for b in range(capb):
                    ps2 = psg2.tile([128, DW], F32, tag="ps2", name=f"ps2_{dc}_{b}")
                    for k in range(KI):
                        nc.tensor.matmul(
                            ps2[:],
                            lhsT=at_sb[:, k * CAP + b * 128: k * CAP + (b + 1) * 128],
                            rhs=dsb[:, k * DW:(k + 1) * DW],
                            start=(k == 0), stop=(k == KI - 1),
                        )
                    yt = y_pool.tile([128, DW], F32, tag="yt")
                    nc.scalar.activation(
                        yt[:], ps2[:],
                        mybir.ActivationFunctionType.Copy,
                        scale=probs_sb[:, soff[j] + b: soff[j] + b + 1],
                    )
                    nc.sync.dma_start(
                        out=y_d[soff[j] + b, :, dc * DW:(dc + 1) * DW], in_=yt[:]
                    )
    nc.compile()
    return nc, names


def _route(indices, token_mask, weights):
    """Replicate the reference's permute/capacity semantics on host."""
    idx = np.asarray(indices).astype(np.int64)
    mask = np.asarray(token_mask).astype(bool)
    w = np.asarray(weights).astype(np.float32)
    flat_e = np.where(mask[:, None], idx, -1).ravel()
    w_flat = np.where(flat_e >= 0, w.ravel(), 0.0).astype(np.float32)
    tok = np.repeat(np.arange(N_TOKENS, dtype=np.int64), TOPK)

    per_expert = []  # (flat_ids, token_ids, weights), flat order, capped at C_REF
    for e in range(N_EXPERTS):
        ids = np.nonzero(flat_e == e)[0][:C_REF]
        per_expert.append((ids, tok[ids], w_flat[ids]))
    return per_expert


def _pack_slots(per_expert):
    """Assign experts to (core, slot) with identical slot capacities per core."""
    needs = [max(1, math.ceil(len(t) / 128)) for _, t, _ in per_expert]
    order = sorted(range(N_EXPERTS), key=lambda e: -needs[e])
    assign = np.empty((NCORE, NSLOT), np.int64)
    caps = []
    for j in range(NSLOT):
        col = order[j * NCORE:(j + 1) * NCORE]
        for m in range(NCORE):
            assign[m, j] = col[m]
        caps.append(max(needs[e] for e in col))
    return assign, tuple(caps)


def _prepare_core_inputs(x, per_expert, gup, down, assign, caps):
    x = np.ascontiguousarray(np.asarray(x, dtype=np.float32))
    gup = np.asarray(gup, dtype=np.float32)
    down = np.asarray(down, dtype=np.float32)
    SB = sum(caps)
    soff = np.concatenate([[0], np.cumsum(caps)]).tolist()
    xt_sizes = [128 * KD * c * 128 for c in caps]
    xt_off = np.concatenate([[0], np.cumsum(xt_sizes)]).tolist()

    in_maps = []
    for m in range(NCORE):
        xt_buf = np.zeros(xt_off[-1], np.float32)
        gup_buf = np.empty((NSLOT, 2, KI, 128, KD * 128), np.float32)
        down_buf = np.empty((NSLOT, NDC, 128, KI * DW), np.float32)
        probs_buf = np.zeros((128, SB), np.float32)
        for j in range(NSLOT):
            CAP = caps[j] * 128
            e = assign[m, j]
            _, toks, ws = per_expert[e]
            n = len(toks)
            xg = np.zeros((CAP, DIM), np.float32)
            xg[:n] = x[toks]
            # [CAP, KD, 128] -> [128(p), KD, CAP]; store halves contiguously
            xt = xg.reshape(CAP, KD, 128).transpose(2, 1, 0)  # [128, KD, CAP]
            half = KD // 2
            blk = 128 * half * CAP
            xt_buf[xt_off[j]: xt_off[j] + blk] = np.ascontiguousarray(xt[:, :half]).ravel()
            xt_buf[xt_off[j] + blk: xt_off[j] + 2 * blk] = np.ascontiguousarray(xt[:, half:]).ravel()
            pw = np.zeros(CAP, np.float32)
            pw[:n] = ws / ALPHA
            probs_buf[:, soff[j]: soff[j] + caps[j]] = pw.reshape(caps[j], 128).T
            for half_gu in (0, 1):
                hm = gup[e, :, half_gu::2]  # [DIM, INTER] gate or up, deinterleaved
                gup_buf[j, half_gu] = (
                    hm.reshape(KD, 128, KI, 128).transpose(2, 1, 0, 3)
                    .reshape(KI, 128, KD * 128)
                )
            dm = down[e]  # [INTER, DIM]
            down_buf[j] = (
                dm.reshape(KI, 128, NDC, DW).transpose(2, 1, 0, 3)
                .reshape(NDC, 128, KI * DW)
            )
        in_maps.append({
            "xt": xt_buf, "gup": gup_buf, "down": down_buf, "probs": probs_buf,
        })
    return in_maps


def _run(inputs: dict, trace: bool = False, tmpdir=None):
    from concourse.bass_utils import run_bass_kernel_spmd

    x = inputs["x"]
    gup = inputs["gate_and_up_projs"]
    down = inputs["down_projs"]

    per_expert = _route(inputs["indices"], inputs["token_mask"], inputs["weights"])
    assign, caps = _pack_slots(per_expert)

    if caps not in _PROG_CACHE:
        _PROG_CACHE[caps] = _build_program(caps)
    nc, names = _PROG_CACHE[caps]

    core_maps = _prepare_core_inputs(x, per_expert, gup, down, assign, caps)
    in_maps = [{names[k]: v for k, v in mm.items()} for mm in core_maps]
    res = run_bass_kernel_spmd(
        nc, in_maps, list(range(NCORE)), trace=trace, tmpdir=tmpdir,
    )

    SB = sum(caps)
    soff = np.concatenate([[0], np.cumsum(caps)]).tolist()
    # stack y rows core-major; expert (m, j) rows at m*SB*128 + soff[j]*128
    ys = [np.asarray(res.results[m][names["y"]]).reshape(SB * 128, DIM)
          for m in range(NCORE)]
    Y = np.concatenate(ys + [np.zeros((1, DIM), np.float32)], axis=0)

    pos = np.full(N_TOKENS * TOPK, NCORE * SB * 128, np.int64)  # default zeros row
    slot_of = {int(assign[m, j]): (m, j) for m in range(NCORE) for j in range(NSLOT)}
    for e in range(N_EXPERTS):
        ids, _, _ = per_expert[e]
        m, j = slot_of[e]
        pos[ids] = m * SB * 128 + soff[j] * 128 + np.arange(len(ids))

    contrib = Y[pos]  # probs already applied on device
    out = contrib.reshape(N_TOKENS, TOPK, DIM).sum(axis=1, dtype=np.float32)
    return out.astype(np.float32), res


def kernel(**inputs) -> np.ndarray:
    out, _ = _run(inputs, trace=False)
    return out



# revision 2
# speedup vs baseline: 1.2425x; 1.2425x over previous
"""MoE grouped-experts kernel for Trainium2 (8 NeuronCores, expert-parallel).

Strategy
--------
Expert-parallel: 32 experts packed onto 8 cores x 4 slots. Routing
(sort-by-expert, capacity truncation at the reference's C=1024) is computed
on host from the tiny `indices` tensor; token rows are gathered per expert,
zero-padded to the slot's streamed length, and pre-transposed so the device
kernel is a pure stream of bf16 matmuls (fp32 PSUM accumulation) with zero
on-device transposes:

  GEMM1 (h^T orientation):  hT[m,c] = sum_k gup[k,m] * xT[k,c]
        stationary = gup tile [128 D-rows, 128 cols-of-2I], moving = xT
  act:  aT = silu(1.702*min(gate,7)) * (clip(up,-7,7)+1)   (the 1/1.702 is
        folded into the routing probs applied in the host-side combine)
  GEMM2 (y^T orientation):  yT[d,c] = sum_k down[k,d] * aT[k,c]
        stationary = natural down chunk [128 I-rows, 128 D-cols],
        moving = aT tokens.  Output leaves the device transposed
        [16 d-chunks, 128, tokens]; the host combine untransposes.

Per-slot token streams are trimmed to the max *actual* expert load in the
slot (padded to a multiple of 16), not to 128-row blocks: streamed columns
beyond an expert's load are zeros on host, flow through as zeros, and are
never gathered.  Routing probs (and the 1/1.702 silu fold) are applied on
the host during the combine, so the device emits unscaled yT.  All operands
are bf16 (PSUM accumulates fp32), halving HBM traffic and SBUF pressure
versus fp32; accuracy stays ~1e-3 relative.
"""

import math
from contextlib import ExitStack

import numpy as np
import ml_dtypes

BF16 = ml_dtypes.bfloat16

N_TOKENS, DIM = 4096, 2048
N_EXPERTS, TOPK, INTER = 32, 4, 1408
ALPHA, LIMIT, LIN_OFFSET = 1.702, 7.0, 1.0

NCORE = 8
NSLOT = N_EXPERTS // NCORE        # expert slots per core = 4
KD = DIM // 128                   # 16 contraction tiles for GEMM1
KI = INTER // 128                 # 11 contraction tiles for GEMM2
NDC = DIM // 128                  # 16 output d-chunks for GEMM2
C_REF = 2 * ((N_TOKENS * TOPK + N_EXPERTS - 1) // N_EXPERTS)  # 1024

_PROG_CACHE: dict = {}


def _groups(lpad: int):
    """Split lpad into PSUM-bank-sized moving groups (<=512, mult of 8)."""
    ng = max(1, math.ceil(lpad / 512))
    per = (lpad // ng // 8) * 8
    sizes = [per] * (ng - 1) + [lpad - per * (ng - 1)]
    assert all(0 < s <= 512 for s in sizes), sizes
    out, off = [], 0
    for s in sizes:
        out.append((off, s))
        off += s
    return out


def _build_program(lpads: tuple):
    import concourse.bacc as bacc
    import concourse.mybir as mybir
    import concourse.tile as tile
    from concourse.alu_op_type import AluOpType

    F32 = mybir.dt.float32
    BF = mybir.dt.bfloat16
    TOT = sum(lpads)
    lmax = max(lpads)
    toff = np.concatenate([[0], np.cumsum(lpads)]).tolist()
    xt_sizes = [128 * KD * lp for lp in lpads]
    xt_off = np.concatenate([[0], np.cumsum(xt_sizes)]).tolist()

    nc = bacc.Bacc(None, target_bir_lowering=False, debug=False)
    with ExitStack() as ctx:
        tc = ctx.enter_context(tile.TileContext(nc))
        dram = ctx.enter_context(tc.tile_pool(name="dram", bufs=1, space="DRAM"))
        xt_d = dram.tile([xt_off[-1]], BF, kind="ExternalInput")
        gup_d = dram.tile([NSLOT, 2, KI, 128, KD * 128], BF, kind="ExternalInput")
        down_d = dram.tile([NSLOT, NDC, 128, KI * 128], BF, kind="ExternalInput")
        y_d = dram.tile([NDC, 128, TOT], F32, kind="ExternalOutput")
        names = {"xt": xt_d.name, "gup": gup_d.name, "down": down_d.name,
                 "y": y_d.name}

        xt_pool = ctx.enter_context(tc.tile_pool(name="xt", bufs=4))
        gup_pool = ctx.enter_context(tc.tile_pool(name="gup", bufs=6))
        down_pool = ctx.enter_context(tc.tile_pool(name="down", bufs=4))
        at_pool = ctx.enter_context(tc.tile_pool(name="at", bufs=2))
        fg_pool = ctx.enter_context(tc.tile_pool(name="fg", bufs=3))
        tmp_pool = ctx.enter_context(tc.tile_pool(name="tmp", bufs=4))
        y_pool = ctx.enter_context(tc.tile_pool(name="yt", bufs=4))
        psg1 = ctx.enter_context(tc.tile_pool(name="psg1", bufs=4, space="PSUM"))
        psg2 = ctx.enter_context(tc.tile_pool(name="psg2", bufs=4, space="PSUM"))

        def load_xt(j):
            """Two half-slabs (k 0..7 / k 8..15) of slot j's xT."""
            lp = lpads[j]
            half_elems = 128 * (KD // 2) * lp
            hs = []
            for h in (0, 1):
                t = xt_pool.tile([128, (KD // 2) * lmax], BF, tag="xt")
                src = xt_d[xt_off[j] + h * half_elems:
                           xt_off[j] + (h + 1) * half_elems]
                nc.sync.dma_start(
                    out=t[:, :(KD // 2) * lp],
                    in_=src.rearrange("(p c) -> p c", p=128),
                )
                hs.append(t)
            return hs

        xt_slabs = load_xt(0)
        for j in range(NSLOT):
            LP = lpads[j]
            groups = _groups(LP)
            xt_h = xt_slabs

            def xt_ap(k, g0, gw, LP=LP, xt_h=xt_h):
                t = xt_h[k // (KD // 2)]
                kk = k % (KD // 2)
                return t[:, kk * LP + g0: kk * LP + g0 + gw]

            at_sb = at_pool.tile([128, KI * lmax], BF, tag="at")

            for i in range(KI):
                for half in (0, 1):  # 0 = gate, 1 = up
                    gsb = gup_pool.tile([128, KD * 128], BF, tag="gup")
                    nc.sync.dma_start(out=gsb[:], in_=gup_d[j, half, i])
                    pss = [psg1.tile([128, 512], F32, tag="ps1",
                                     name=f"ps1_{j}_{i}_{half}_{gi}")
                           for gi in range(len(groups))]
                    for k in range(KD):
                        for gi, (g0, gw) in enumerate(groups):
                            nc.tensor.matmul(
                                pss[gi][:, :gw],
                                lhsT=gsb[:, k * 128:(k + 1) * 128],
                                rhs=xt_ap(k, g0, gw),
                                start=(k == 0), stop=(k == KD - 1),
                            )
                    for gi, (g0, gw) in enumerate(groups):
                        ps = pss[gi]
                        if half == 0:
                            t0 = tmp_pool.tile([128, 512], F32, tag="t0")
                            nc.vector.tensor_scalar_min(t0[:, :gw], ps[:, :gw], LIMIT)
                            fg = fg_pool.tile([128, 512], F32, tag="fg")
                            nc.scalar.activation(
                                fg[:, :gw], t0[:, :gw],
                                mybir.ActivationFunctionType.Silu, scale=ALPHA,
                            )
                            if gi == 0:
                                fgs = [fg]
                            else:
                                fgs.append(fg)
                        else:
                            uc = tmp_pool.tile([128, 512], F32, tag="uc")
                            nc.vector.tensor_scalar(
                                uc[:, :gw], ps[:, :gw], LIMIT, -LIMIT,
                                AluOpType.min, AluOpType.max,
                            )
                            # aT = (clip(up)+1) * silu(1.702*min(gate,7))
                            nc.vector.scalar_tensor_tensor(
                                at_sb[:, i * LP + g0: i * LP + g0 + gw],
                                uc[:, :gw], LIN_OFFSET, fgs[gi][:, :gw],
                                AluOpType.add, AluOpType.mult,
                            )

            if j + 1 < NSLOT:  # prefetch next slot's tokens under GEMM2
                xt_slabs = load_xt(j + 1)

            for dc in range(NDC):
                dsb = down_pool.tile([128, KI * 128], BF, tag="down")
                nc.sync.dma_start(out=dsb[:], in_=down_d[j, dc])
                ps2s = [psg2.tile([128, 512], F32, tag="ps2",
                                  name=f"ps2_{j}_{dc}_{gi}")
                        for gi in range(len(groups))]
                for k in range(KI):
                    for gi, (g0, gw) in enumerate(groups):
                        nc.tensor.matmul(
                            ps2s[gi][:, :gw],
                            lhsT=dsb[:, k * 128:(k + 1) * 128],
                            rhs=at_sb[:, k * LP + g0: k * LP + g0 + gw],
                            start=(k == 0), stop=(k == KI - 1),
                        )
                for gi, (g0, gw) in enumerate(groups):
                    yt = y_pool.tile([128, 512], F32, tag="yt")
                    nc.scalar.activation(
                        yt[:, :gw], ps2s[gi][:, :gw],
                        mybir.ActivationFunctionType.Copy,
                    )
                    nc.sync.dma_start(
                        out=y_d[dc, :, toff[j] + g0: toff[j] + g0 + gw],
                        in_=yt[:, :gw],
                    )
    nc.compile()
    return nc, names


def _route(indices, token_mask, weights):
    """Replicate the reference's permute/capacity semantics on host."""
    idx = np.asarray(indices).astype(np.int64)
    mask = np.asarray(token_mask).astype(bool)
    w = np.asarray(weights).astype(np.float32)
    flat_e = np.where(mask[:, None], idx, -1).ravel()
    w_flat = np.where(flat_e >= 0, w.ravel(), 0.0).astype(np.float32)
    tok = np.repeat(np.arange(N_TOKENS, dtype=np.int64), TOPK)

    per_expert = []  # (flat_ids, token_ids, weights), flat order, capped at C_REF
    for e in range(N_EXPERTS):
        ids = np.nonzero(flat_e == e)[0][:C_REF]
        per_expert.append((ids, tok[ids], w_flat[ids]))
    return per_expert


def _pack_slots(per_expert):
    """Assign experts to (core, slot); slot stream length = max load in slot."""
    loads = [len(t) for _, t, _ in per_expert]
    order = sorted(range(N_EXPERTS), key=lambda e: -loads[e])
    assign = np.empty((NCORE, NSLOT), np.int64)
    lpads = []
    for j in range(NSLOT):
        col = order[j * NCORE:(j + 1) * NCORE]
        for m in range(NCORE):
            assign[m, j] = col[m]
        lmax = max(loads[e] for e in col)
        lpads.append(max(16, ((lmax + 15) // 16) * 16))
    return assign, tuple(lpads)


def _prepare_core_inputs(x, per_expert, gup, down, assign, lpads):
    x16 = np.ascontiguousarray(np.asarray(x, dtype=np.float32)).astype(BF16)
    gup16 = np.asarray(gup, dtype=np.float32).astype(BF16)
    down16 = np.asarray(down, dtype=np.float32).astype(BF16)
    xt_sizes = [128 * KD * lp for lp in lpads]
    xt_off = np.concatenate([[0], np.cumsum(xt_sizes)]).tolist()

    in_maps = []
    for m in range(NCORE):
        xt_buf = np.zeros(xt_off[-1], BF16)
        gup_buf = np.empty((NSLOT, 2, KI, 128, KD * 128), BF16)
        down_buf = np.empty((NSLOT, NDC, 128, KI * 128), BF16)
        for j in range(NSLOT):
            LP = lpads[j]
            e = assign[m, j]
            _, toks, _ = per_expert[e]
            n = len(toks)
            xg = np.zeros((LP, DIM), BF16)
            xg[:n] = x16[toks]
            # [LP, KD, 128] -> [128(p), KD, LP]; store halves contiguously
            xt = xg.reshape(LP, KD, 128).transpose(2, 1, 0)  # [128, KD, LP]
            half = KD // 2
            blk = 128 * half * LP
            xt_buf[xt_off[j]: xt_off[j] + blk] = np.ascontiguousarray(xt[:, :half]).ravel()
            xt_buf[xt_off[j] + blk: xt_off[j] + 2 * blk] = np.ascontiguousarray(xt[:, half:]).ravel()
            for half_gu in (0, 1):
                hm = gup16[e, :, half_gu::2]  # [DIM, INTER] gate or up, deinterleaved
                gup_buf[j, half_gu] = (
                    hm.reshape(KD, 128, KI, 128).transpose(2, 1, 0, 3)
                    .reshape(KI, 128, KD * 128)
                )
            dm = down16[e]  # [INTER, DIM] natural layout, chunked by 128 d-cols
            down_buf[j] = (
                dm.reshape(KI, 128, NDC, 128).transpose(2, 1, 0, 3)
                .reshape(NDC, 128, KI * 128)
            )
        in_maps.append({"xt": xt_buf, "gup": gup_buf, "down": down_buf})
    return in_maps


def _run(inputs: dict, trace: bool = False, tmpdir=None):
    from concourse.bass_utils import run_bass_kernel_spmd

    x = inputs["x"]
    gup = inputs["gate_and_up_projs"]
    down = inputs["down_projs"]

    per_expert = _route(inputs["indices"], inputs["token_mask"], inputs["weights"])
    assign, lpads = _pack_slots(per_expert)

    if lpads not in _PROG_CACHE:
        _PROG_CACHE[lpads] = _build_program(lpads)
    nc, names = _PROG_CACHE[lpads]

    core_maps = _prepare_core_inputs(x, per_expert, gup, down, assign, lpads)
    in_maps = [{names[k]: v for k, v in mm.items()} for mm in core_maps]
    res = run_bass_kernel_spmd(
        nc, in_maps, list(range(NCORE)), trace=trace, tmpdir=tmpdir,
    )

    TOT = sum(lpads)
    toff = np.concatenate([[0], np.cumsum(lpads)]).tolist()
    # yT per core: [NDC, 128, TOT] -> [DIM, TOT]
    Y = np.stack([np.asarray(res.results[m][names["y"]]).reshape(DIM, TOT)
                  for m in range(NCORE)])  # [NCORE, DIM, TOT]

    T = N_TOKENS * TOPK
    core_of = np.zeros(T, np.int64)
    col_of = np.zeros(T, np.int64)
    wgt = np.zeros(T, np.float32)
    slot_of = {int(assign[m, j]): (m, j) for m in range(NCORE) for j in range(NSLOT)}
    for e in range(N_EXPERTS):
        ids, _, ws = per_expert[e]
        m, j = slot_of[e]
        core_of[ids] = m
        col_of[ids] = toff[j] + np.arange(len(ids))
        wgt[ids] = ws / ALPHA          # fold silu(a*g)/a into the combine
    contrib = Y[core_of, :, col_of]    # [T, DIM]
    out = (contrib * wgt[:, None]).reshape(N_TOKENS, TOPK, DIM).sum(axis=1)
    return np.ascontiguousarray(out, dtype=np.float32), res


def kernel(**inputs) -> np.ndarray:
    out, _ = _run(inputs, trace=False)
    return out


# revision 4
# speedup vs baseline: 1.2727x; 1.0243x over previous
"""MoE grouped-experts kernel for Trainium2 (8 NeuronCores, expert-parallel).

Strategy
--------
Expert-parallel: 32 experts packed onto 8 cores x 4 slots. Routing
(sort-by-expert, capacity truncation at the reference's C=1024) is computed
on host from the tiny `indices` tensor; token rows are gathered per expert,
zero-padded to the slot's streamed length, and pre-transposed so the device
kernel is a pure stream of bf16 matmuls (fp32 PSUM accumulation) with zero
on-device transposes:

  GEMM1 (h^T orientation):  hT[m,c] = sum_k gup[k,m] * xT[k,c]
        stationary = gup tile [128 D-rows, 128 cols-of-2I], moving = xT
  act:  aT = silu(1.702*min(gate,7)) * (clip(up,-7,7)+1)   (the 1/1.702 is
        folded into the routing probs applied in the host-side combine)
  GEMM2 (y^T orientation):  yT[d,c] = sum_k down[k,d] * aT[k,c]
        stationary = natural down chunk [128 I-rows, 128 D-cols],
        moving = aT tokens.  Output leaves the device transposed
        [16 d-chunks, 128, tokens]; the host combine untransposes.

Per-slot token streams are trimmed to the max *actual* expert load in the
slot (padded to a multiple of 16), not to 128-row blocks: streamed columns
beyond an expert's load are zeros on host, flow through as zeros, and are
never gathered.  Routing probs (and the 1/1.702 silu fold) are applied on
the host during the combine, so the device emits unscaled yT.  All operands
are bf16 (PSUM accumulates fp32), halving HBM traffic and SBUF pressure
versus fp32.

DMA pacing: the sync ring is a single FIFO fanned over 16 engines, so
every transfer is issued in need-order and sized so nothing blocks a
latency-critical load: xT is split per contraction tile (139 KB), gup
stationaries are issued two (i,half) steps ahead across slot boundaries,
and the next slot's xT is paced between GEMM2's down-chunk loads.
"""

import math
from contextlib import ExitStack

import numpy as np
import ml_dtypes

BF16 = ml_dtypes.bfloat16

N_TOKENS, DIM = 4096, 2048
N_EXPERTS, TOPK, INTER = 32, 4, 1408
ALPHA, LIMIT, LIN_OFFSET = 1.702, 7.0, 1.0

NCORE = 8
NSLOT = N_EXPERTS // NCORE        # expert slots per core = 4
KD = DIM // 128                   # 16 contraction tiles for GEMM1
KI = INTER // 128                 # 11 contraction tiles for GEMM2
NDC = DIM // 128                  # 16 output d-chunks for GEMM2
C_REF = 2 * ((N_TOKENS * TOPK + N_EXPERTS - 1) // N_EXPERTS)  # 1024

_PROG_CACHE: dict = {}


def _groups(lpad: int):
    """Split lpad into PSUM-bank-sized moving groups (<=512, mult of 8)."""
    ng = max(1, math.ceil(lpad / 512))
    per = (lpad // ng // 8) * 8
    sizes = [per] * (ng - 1) + [lpad - per * (ng - 1)]
    assert all(0 < s <= 512 for s in sizes), sizes
    out, off = [], 0
    for s in sizes:
        out.append((off, s))
        off += s
    return out


def _build_program(lpads: tuple):
    import concourse.bacc as bacc
    import concourse.mybir as mybir
    import concourse.tile as tile
    from concourse.alu_op_type import AluOpType

    F32 = mybir.dt.float32
    BF = mybir.dt.bfloat16
    TOT = sum(lpads)
    lmax = max(lpads)
    toff = np.concatenate([[0], np.cumsum(lpads)]).tolist()
    xt_sizes = [KD * 128 * lp for lp in lpads]
    xt_off = np.concatenate([[0], np.cumsum(xt_sizes)]).tolist()

    nc = bacc.Bacc(None, target_bir_lowering=False, debug=False)
    with ExitStack() as ctx:
        tc = ctx.enter_context(tile.TileContext(nc))
        dram = ctx.enter_context(tc.tile_pool(name="dram", bufs=1, space="DRAM"))
        xt_d = dram.tile([xt_off[-1]], BF, kind="ExternalInput")
        gup_d = dram.tile([NSLOT, 2, KI, 128, KD * 128], BF, kind="ExternalInput")
        down_d = dram.tile([NSLOT, NDC, 128, KI * 128], BF, kind="ExternalInput")
        y_d = dram.tile([NDC, 128, TOT], F32, kind="ExternalOutput")
        names = {"xt": xt_d.name, "gup": gup_d.name, "down": down_d.name,
                 "y": y_d.name}

        xt_pool = ctx.enter_context(tc.tile_pool(name="xt", bufs=2 * KD + 2))
        gup_pool = ctx.enter_context(tc.tile_pool(name="gup", bufs=6))
        down_pool = ctx.enter_context(tc.tile_pool(name="down", bufs=4))
        at_pool = ctx.enter_context(tc.tile_pool(name="at", bufs=2))
        fg_pool = ctx.enter_context(tc.tile_pool(name="fg", bufs=3))
        tmp_pool = ctx.enter_context(tc.tile_pool(name="tmp", bufs=4))
        y_pool = ctx.enter_context(tc.tile_pool(name="yt", bufs=4))
        psg1 = ctx.enter_context(tc.tile_pool(name="psg1", bufs=4, space="PSUM"))
        psg2 = ctx.enter_context(tc.tile_pool(name="psg2", bufs=4, space="PSUM"))

        def load_xt_k(j, k):
            """One contraction tile [128, lp] of slot j's xT."""
            lp = lpads[j]
            t = xt_pool.tile([128, lmax], BF, tag="xt")
            src = xt_d[xt_off[j] + k * 128 * lp: xt_off[j] + (k + 1) * 128 * lp]
            nc.sync.dma_start(out=t[:, :lp],
                              in_=src.rearrange("(p c) -> p c", p=128))
            return t

        # flat (j, i, half) order of gup stationary loads, prefetched depth-2
        gsteps = [(j, i, half)
                  for j in range(NSLOT) for i in range(KI) for half in (0, 1)]
        gup_tiles: dict = {}

        def issue_gup(s):
            if s < len(gsteps):
                j, i, half = gsteps[s]
                t = gup_pool.tile([128, KD * 128], BF, tag="gup")
                nc.sync.dma_start(out=t[:], in_=gup_d[j, half, i])
                gup_tiles[s] = t

        issue_gup(0)
        xt_tiles = [load_xt_k(0, k) for k in range(4)]
        issue_gup(1)
        xt_tiles += [load_xt_k(0, k) for k in range(4, KD)]

        down_tiles: dict = {}

        def issue_down(j, dc):
            t = down_pool.tile([128, KI * 128], BF, tag="down")
            nc.sync.dma_start(out=t[:], in_=down_d[j, dc])
            down_tiles[dc] = t

        for j in range(NSLOT):
            LP = lpads[j]
            groups = _groups(LP)
            xt_h = xt_tiles
            next_xt: list = []

            at_sb = at_pool.tile([128, KI * lmax], BF, tag="at")

            for i in range(KI):
                for half in (0, 1):  # 0 = gate, 1 = up
                    s = (j * KI + i) * 2 + half
                    issue_gup(s + 2)
                    if i == KI - 2 and half == 1:
                        issue_down(j, 0)
                        issue_down(j, 1)
                    gsb = gup_tiles.pop(s)
                    pss = [psg1.tile([128, 512], F32, tag="ps1",
                                     name=f"ps1_{j}_{i}_{half}_{gi}")
                           for gi in range(len(groups))]
                    for k in range(KD):
                        for gi, (g0, gw) in enumerate(groups):
                            nc.tensor.matmul(
                                pss[gi][:, :gw],
                                lhsT=gsb[:, k * 128:(k + 1) * 128],
                                rhs=xt_h[k][:, g0:g0 + gw],
                                start=(k == 0), stop=(k == KD - 1),
                            )
                    for gi, (g0, gw) in enumerate(groups):
                        ps = pss[gi]
                        if half == 0:
                            t0 = tmp_pool.tile([128, 512], F32, tag="t0")
                            nc.vector.tensor_scalar_min(t0[:, :gw], ps[:, :gw], LIMIT)
                            fg = fg_pool.tile([128, 512], F32, tag="fg")
                            nc.scalar.activation(
                                fg[:, :gw], t0[:, :gw],
                                mybir.ActivationFunctionType.Silu, scale=ALPHA,
                            )
                            if gi == 0:
                                fgs = [fg]
                            else:
                                fgs.append(fg)
                        else:
                            uc = tmp_pool.tile([128, 512], F32, tag="uc")
                            nc.vector.tensor_scalar(
                                uc[:, :gw], ps[:, :gw], LIMIT, -LIMIT,
                                AluOpType.min, AluOpType.max,
                            )
                            # aT = (clip(up)+1) * silu(1.702*min(gate,7))
                            nc.vector.scalar_tensor_tensor(
                                at_sb[:, i * LP + g0: i * LP + g0 + gw],
                                uc[:, :gw], LIN_OFFSET, fgs[gi][:, :gw],
                                AluOpType.add, AluOpType.mult,
                            )

            for dc in range(NDC):
                if dc + 2 < NDC:
                    issue_down(j, dc + 2)
                if j + 1 < NSLOT and dc < KD:  # pace next slot's xT
                    next_xt.append(load_xt_k(j + 1, dc))
                dsb = down_tiles.pop(dc)
                ps2s = [psg2.tile([128, 512], F32, tag="ps2",
                                  name=f"ps2_{j}_{dc}_{gi}")
                        for gi in range(len(groups))]
                for k in range(KI):
                    for gi, (g0, gw) in enumerate(groups):
                        nc.tensor.matmul(
                            ps2s[gi][:, :gw],
                            lhsT=dsb[:, k * 128:(k + 1) * 128],
                            rhs=at_sb[:, k * LP + g0: k * LP + g0 + gw],
                            start=(k == 0), stop=(k == KI - 1),
                        )
                for gi, (g0, gw) in enumerate(groups):
                    yt = y_pool.tile([128, 512], F32, tag="yt")
                    nc.scalar.activation(
                        yt[:, :gw], ps2s[gi][:, :gw],
                        mybir.ActivationFunctionType.Copy,
                    )
                    nc.sync.dma_start(
                        out=y_d[dc, :, toff[j] + g0: toff[j] + g0 + gw],
                        in_=yt[:, :gw],
                    )
            xt_tiles = next_xt
    nc.compile()
    return nc, names


def _route(indices, token_mask, weights):
    """Replicate the reference's permute/capacity semantics on host."""
    idx = np.asarray(indices).astype(np.int64)
    mask = np.asarray(token_mask).astype(bool)
    w = np.asarray(weights).astype(np.float32)
    flat_e = np.where(mask[:, None], idx, -1).ravel()
    w_flat = np.where(flat_e >= 0, w.ravel(), 0.0).astype(np.float32)
    tok = np.repeat(np.arange(N_TOKENS, dtype=np.int64), TOPK)

    per_expert = []  # (flat_ids, token_ids, weights), flat order, capped at C_REF
    for e in range(N_EXPERTS):
        ids = np.nonzero(flat_e == e)[0][:C_REF]
        per_expert.append((ids, tok[ids], w_flat[ids]))
    return per_expert


def _pack_slots(per_expert):
    """Assign experts to (core, slot); slot stream length = max load in slot."""
    loads = [len(t) for _, t, _ in per_expert]
    order = sorted(range(N_EXPERTS), key=lambda e: -loads[e])
    assign = np.empty((NCORE, NSLOT), np.int64)
    lpads = []
    for j in range(NSLOT):
        col = order[j * NCORE:(j + 1) * NCORE]
        for m in range(NCORE):
            assign[m, j] = col[m]
        lmax = max(loads[e] for e in col)
        lpads.append(max(16, ((lmax + 15) // 16) * 16))
    return assign, tuple(lpads)


def _prepare_core_inputs(x, per_expert, gup, down, assign, lpads):
    x16 = np.ascontiguousarray(np.asarray(x, dtype=np.float32)).astype(BF16)
    gup16 = np.asarray(gup, dtype=np.float32).astype(BF16)
    down16 = np.asarray(down, dtype=np.float32).astype(BF16)
    xt_sizes = [KD * 128 * lp for lp in lpads]
    xt_off = np.concatenate([[0], np.cumsum(xt_sizes)]).tolist()

    in_maps = []
    for m in range(NCORE):
        xt_buf = np.zeros(xt_off[-1], BF16)
        gup_buf = np.empty((NSLOT, 2, KI, 128, KD * 128), BF16)
        down_buf = np.empty((NSLOT, NDC, 128, KI * 128), BF16)
        for j in range(NSLOT):
            LP = lpads[j]
            e = assign[m, j]
            _, toks, _ = per_expert[e]
            n = len(toks)
            xg = np.zeros((LP, DIM), BF16)
            xg[:n] = x16[toks]
            # [LP, KD, 128] -> k-major [KD, 128(p), LP]
            xt = xg.reshape(LP, KD, 128).transpose(1, 2, 0)
            xt_buf[xt_off[j]: xt_off[j + 1]] = np.ascontiguousarray(xt).ravel()
            for half_gu in (0, 1):
                hm = gup16[e, :, half_gu::2]  # [DIM, INTER] gate or up, deinterleaved
                gup_buf[j, half_gu] = (
                    hm.reshape(KD, 128, KI, 128).transpose(2, 1, 0, 3)
                    .reshape(KI, 128, KD * 128)
                )
            dm = down16[e]  # [INTER, DIM] natural layout, chunked by 128 d-cols
            down_buf[j] = (
                dm.reshape(KI, 128, NDC, 128).transpose(2, 1, 0, 3)
                .reshape(NDC, 128, KI * 128)
            )
        in_maps.append({"xt": xt_buf, "gup": gup_buf, "down": down_buf})
    return in_maps


def _run(inputs: dict, trace: bool = False, tmpdir=None):
    from concourse.bass_utils import run_bass_kernel_spmd

    x = inputs["x"]
    gup = inputs["gate_and_up_projs"]
    down = inputs["down_projs"]

    per_expert = _route(inputs["indices"], inputs["token_mask"], inputs["weights"])
    assign, lpads = _pack_slots(per_expert)

    if lpads not in _PROG_CACHE:
        _PROG_CACHE[lpads] = _build_program(lpads)
    nc, names = _PROG_CACHE[lpads]

    core_maps = _prepare_core_inputs(x, per_expert, gup, down, assign, lpads)
    in_maps = [{names[k]: v for k, v in mm.items()} for mm in core_maps]
    res = run_bass_kernel_spmd(
        nc, in_maps, list(range(NCORE)), trace=trace, tmpdir=tmpdir,
    )

    TOT = sum(lpads)
    toff = np.concatenate([[0], np.cumsum(lpads)]).tolist()
    # yT per core: [NDC, 128, TOT] -> [DIM, TOT]
    Y = np.stack([np.asarray(res.results[m][names["y"]]).reshape(DIM, TOT)
                  for m in range(NCORE)])  # [NCORE, DIM, TOT]

    T = N_TOKENS * TOPK
    core_of = np.zeros(T, np.int64)
    col_of = np.zeros(T, np.int64)
    wgt = np.zeros(T, np.float32)
    slot_of = {int(assign[m, j]): (m, j) for m in range(NCORE) for j in range(NSLOT)}
    for e in range(N_EXPERTS):
        ids, _, ws = per_expert[e]
        m, j = slot_of[e]
        core_of[ids] = m
        col_of[ids] = toff[j] + np.arange(len(ids))
        wgt[ids] = ws / ALPHA          # fold silu(a*g)/a into the combine
    contrib = Y[core_of, :, col_of]    # [T, DIM]
    out = (contrib * wgt[:, None]).reshape(N_TOKENS, TOPK, DIM).sum(axis=1)
    return np.ascontiguousarray(out, dtype=np.float32), res


def kernel(**inputs) -> np.ndarray:
    out, _ = _run(inputs, trace=False)
    return out


# revision 11
# speedup vs baseline: 1.3484x; 1.0595x over previous
"""MoE grouped-experts kernel for Trainium2 (8 NeuronCores, expert-parallel).

Strategy
--------
Expert-parallel: 32 experts packed onto 8 cores x 4 slots. Routing
(sort-by-expert, capacity truncation at the reference's C=1024) is computed
on host from the tiny `indices` tensor; token rows are gathered per expert,
zero-padded to the slot's streamed length, and pre-transposed so the device
kernel is a pure stream of bf16 matmuls (fp32 PSUM accumulation) with zero
on-device transposes:

  GEMM1 (h^T orientation):  hT[m,c] = sum_k gup[k,m] * xT[k,c]
        stationary = gup tile [128 D-rows, 128 cols-of-2I], moving = xT
  act:  aT = silu(1.702*min(gate,7)) * (clip(up,-7,7)+1)   (the 1/1.702 is
        folded into the routing probs applied in the host-side combine)
  GEMM2 (y^T orientation):  yT[d,c] = sum_k down[k,d] * aT[k,c]
        stationary = natural down chunk [128 I-rows, 128 D-cols],
        moving = aT tokens.  Output leaves the device transposed
        [16 d-chunks, 128, tokens]; the host combine untransposes.

Per-slot token streams are trimmed to the max *actual* expert load in the
slot (padded to a multiple of 16), not to 128-row blocks: streamed columns
beyond an expert's load are zeros on host, flow through as zeros, and are
never gathered.  Routing probs (and the 1/1.702 silu fold) are applied on
the host during the combine, so the device emits unscaled yT.  All operands
are bf16 (PSUM accumulates fp32), halving HBM traffic and SBUF pressure
versus fp32.

DMA pacing: the sync ring is a single FIFO fanned over 16 engines, so
every transfer is issued in need-order and sized so nothing blocks a
latency-critical load: xT is split per contraction tile (139 KB), gup
stationaries are issued two (i,half) steps ahead across slot boundaries,
and the next slot's xT is paced between GEMM2's down-chunk loads.
"""

import math
from contextlib import ExitStack

import numpy as np
import ml_dtypes

BF16 = ml_dtypes.bfloat16

N_TOKENS, DIM = 4096, 2048
N_EXPERTS, TOPK, INTER = 32, 4, 1408
ALPHA, LIMIT, LIN_OFFSET = 1.702, 7.0, 1.0

NCORE = 8
NSLOT = N_EXPERTS // NCORE        # expert slots per core = 4
KD = DIM // 128                   # 16 contraction tiles for GEMM1
KI = INTER // 128                 # 11 contraction tiles for GEMM2
NDC = DIM // 128                  # 16 output d-chunks for GEMM2
C_REF = 2 * ((N_TOKENS * TOPK + N_EXPERTS - 1) // N_EXPERTS)  # 1024

_PROG_CACHE: dict = {}


def _groups(lpad: int):
    """Split lpad into PSUM-bank-sized moving groups (<=512, mult of 2)."""
    ng = max(1, math.ceil(lpad / 512))
    per = (lpad // ng // 2) * 2
    sizes = [per] * (ng - 1) + [lpad - per * (ng - 1)]
    assert all(0 < s <= 512 for s in sizes), sizes
    out, off = [], 0
    for s in sizes:
        out.append((off, s))
        off += s
    return out


def _build_program(lpads: tuple):
    import concourse.bacc as bacc
    import concourse.mybir as mybir
    import concourse.tile as tile
    from concourse.alu_op_type import AluOpType

    F32 = mybir.dt.float32
    BF = mybir.dt.bfloat16
    TOT = sum(lpads)
    lmax = max(lpads)
    toff = np.concatenate([[0], np.cumsum(lpads)]).tolist()
    xt_sizes = [KD * 128 * lp for lp in lpads]
    xt_off = np.concatenate([[0], np.cumsum(xt_sizes)]).tolist()

    nc = bacc.Bacc(None, target_bir_lowering=False, debug=False)
    with ExitStack() as ctx:
        tc = ctx.enter_context(tile.TileContext(nc))
        dram = ctx.enter_context(tc.tile_pool(name="dram", bufs=1, space="DRAM"))
        xt_d = dram.tile([xt_off[-1]], BF, kind="ExternalInput")
        gup_d = dram.tile([NSLOT, 2, KI, 128, KD * 128], BF, kind="ExternalInput")
        down_d = dram.tile([NSLOT, NDC, 128, KI * 128], BF, kind="ExternalInput")
        y_d = dram.tile([NDC, 128, TOT], F32, kind="ExternalOutput")
        names = {"xt": xt_d.name, "gup": gup_d.name, "down": down_d.name,
                 "y": y_d.name}

        xt_pool = ctx.enter_context(tc.tile_pool(name="xt", bufs=2 * KD + 2))
        gup_pool = ctx.enter_context(tc.tile_pool(name="gup", bufs=6))
        gup0_pool = ctx.enter_context(tc.tile_pool(name="gup0", bufs=2))
        down_pool = ctx.enter_context(tc.tile_pool(name="down", bufs=4))
        at_pool = ctx.enter_context(tc.tile_pool(name="at", bufs=2))
        fg_pool = ctx.enter_context(tc.tile_pool(name="fg", bufs=3))
        tmp_pool = ctx.enter_context(tc.tile_pool(name="tmp", bufs=4))
        y_pool = ctx.enter_context(tc.tile_pool(name="yt", bufs=4))
        psg = ctx.enter_context(tc.tile_pool(name="psg", bufs=8, space="PSUM"))

        def load_xt_k(j, k):
            """One contraction tile [128, lp] of slot j's xT."""
            lp = lpads[j]
            t = xt_pool.tile([128, lmax], BF, tag="xt")
            src = xt_d[xt_off[j] + k * 128 * lp: xt_off[j] + (k + 1) * 128 * lp]
            nc.sync.dma_start(out=t[:, :lp],
                              in_=src.rearrange("(p c) -> p c", p=128))
            return t

        # flat (j, i, half) order of gup stationary loads, prefetched depth-2
        gsteps = [(j, i, half)
                  for j in range(NSLOT) for i in range(KI) for half in (0, 1)]
        gup_tiles: dict = {}

        def issue_gup(s):
            if s < len(gsteps):
                j, i, half = gsteps[s]
                t = gup_pool.tile([128, KD * 128], BF, tag="gup")
                nc.sync.dma_start(out=t[:], in_=gup_d[j, half, i])
                gup_tiles[s] = t

        # first stationary is split in two k-halves so matmul k=0 starts early
        g0a = gup0_pool.tile([128, (KD // 2) * 128], BF, tag="gup0")
        nc.sync.dma_start(out=g0a[:], in_=gup_d[0, 0, 0, :, :(KD // 2) * 128])
        xt_tiles = [load_xt_k(0, 0)]
        g0b = gup0_pool.tile([128, (KD // 2) * 128], BF, tag="gup0")
        nc.sync.dma_start(out=g0b[:], in_=gup_d[0, 0, 0, :, (KD // 2) * 128:])
        gup_tiles[0] = (g0a, g0b)
        xt_tiles += [load_xt_k(0, k) for k in range(1, 4)]
        issue_gup(1)
        xt_tiles += [load_xt_k(0, k) for k in range(4, KD)]

        down_tiles: dict = {}

        def issue_down(j, dc):
            t = down_pool.tile([128, KI * 128], BF, tag="down")
            nc.sync.dma_start(out=t[:], in_=down_d[j, dc])
            down_tiles[dc] = t

        for j in range(NSLOT):
            LP = lpads[j]
            groups = _groups(LP)
            xt_h = xt_tiles
            next_xt: list = []

            at_sb = at_pool.tile([128, KI * lmax], BF, tag="at")

            for i in range(KI):
                for half in (0, 1):  # 0 = gate, 1 = up
                    s = (j * KI + i) * 2 + half
                    issue_gup(s + 2)
                    if i == KI - 2 and half == 1:
                        issue_down(j, 0)
                        issue_down(j, 1)
                    gsb = gup_tiles.pop(s)
                    if isinstance(gsb, tuple):
                        def g_ap(k, gsb=gsb):
                            h = KD // 2
                            return gsb[k // h][:, (k % h) * 128:(k % h + 1) * 128]
                    else:
                        def g_ap(k, gsb=gsb):
                            return gsb[:, k * 128:(k + 1) * 128]
                    pss = [psg.tile([128, 512], F32, tag="ps",
                                    name=f"ps1_{j}_{i}_{half}_{gi}")
                           for gi in range(len(groups))]
                    for k in range(KD):
                        for gi, (g0, gw) in enumerate(groups):
                            nc.tensor.matmul(
                                pss[gi][:, :gw],
                                lhsT=g_ap(k),
                                rhs=xt_h[k][:, g0:g0 + gw],
                                start=(k == 0), stop=(k == KD - 1),
                            )
                    for gi, (g0, gw) in enumerate(groups):
                        ps = pss[gi]
                        if half == 0:
                            t0 = tmp_pool.tile([128, 512], F32, tag="t0")
                            nc.vector.tensor_scalar_min(t0[:, :gw], ps[:, :gw], LIMIT)
                            fg = fg_pool.tile([128, 512], F32, tag="fg")
                            nc.scalar.activation(
                                fg[:, :gw], t0[:, :gw],
                                mybir.ActivationFunctionType.Silu, scale=ALPHA,
                            )
                            if gi == 0:
                                fgs = [fg]
                            else:
                                fgs.append(fg)
                        else:
                            uc = tmp_pool.tile([128, 512], F32, tag="uc")
                            nc.vector.tensor_scalar(
                                uc[:, :gw], ps[:, :gw], LIMIT, -LIMIT,
                                AluOpType.min, AluOpType.max,
                            )
                            # aT = (clip(up)+1) * silu(1.702*min(gate,7))
                            nc.vector.scalar_tensor_tensor(
                                at_sb[:, i * LP + g0: i * LP + g0 + gw],
                                uc[:, :gw], LIN_OFFSET, fgs[gi][:, :gw],
                                AluOpType.add, AluOpType.mult,
                            )

            for dc in range(NDC):
                if dc + 2 < NDC:
                    issue_down(j, dc + 2)
                if j + 1 < NSLOT and dc < KD:  # pace next slot's xT
                    next_xt.append(load_xt_k(j + 1, dc))
                dsb = down_tiles.pop(dc)
                ps2s = [psg.tile([128, 512], F32, tag="ps",
                                 name=f"ps2_{j}_{dc}_{gi}")
                        for gi in range(len(groups))]
                for k in range(KI):
                    for gi, (g0, gw) in enumerate(groups):
                        nc.tensor.matmul(
                            ps2s[gi][:, :gw],
                            lhsT=dsb[:, k * 128:(k + 1) * 128],
                            rhs=at_sb[:, k * LP + g0: k * LP + g0 + gw],
                            start=(k == 0), stop=(k == KI - 1),
                        )
                for gi, (g0, gw) in enumerate(groups):
                    yt = y_pool.tile([128, 512], F32, tag="yt")
                    nc.scalar.activation(
                        yt[:, :gw], ps2s[gi][:, :gw],
                        mybir.ActivationFunctionType.Copy,
                    )
                    nc.sync.dma_start(
                        out=y_d[dc, :, toff[j] + g0: toff[j] + g0 + gw],
                        in_=yt[:, :gw],
                    )
            xt_tiles = next_xt
    nc.compile()
    return nc, names


def _route(indices, token_mask, weights):
    """Replicate the reference's permute/capacity semantics on host."""
    idx = np.asarray(indices).astype(np.int64)
    mask = np.asarray(token_mask).astype(bool)
    w = np.asarray(weights).astype(np.float32)
    flat_e = np.where(mask[:, None], idx, -1).ravel()
    w_flat = np.where(flat_e >= 0, w.ravel(), 0.0).astype(np.float32)
    tok = np.repeat(np.arange(N_TOKENS, dtype=np.int64), TOPK)

    per_expert = []  # (first flat_ids, unique token_ids, merged weights)
    for e in range(N_EXPERTS):
        ids = np.nonzero(flat_e == e)[0][:C_REF]
        # a token routed to the same expert k times contributes (w1+..+wk)*y;
        # merge duplicates so each (token, expert) pair is computed once
        ut, first_idx, inv = np.unique(tok[ids], return_index=True,
                                       return_inverse=True)
        uw = np.bincount(inv, weights=w_flat[ids]).astype(np.float32)
        per_expert.append((ids[first_idx], ut, uw))
    return per_expert


def _pack_slots(per_expert):
    """Assign experts to (core, slot); slot stream length = max load in slot."""
    loads = [len(t) for _, t, _ in per_expert]
    order = sorted(range(N_EXPERTS), key=lambda e: -loads[e])
    assign = np.empty((NCORE, NSLOT), np.int64)
    lpads = []
    for j in range(NSLOT):
        col = order[j * NCORE:(j + 1) * NCORE]
        for m in range(NCORE):
            assign[m, j] = col[m]
        lmax = max(loads[e] for e in col)
        lpads.append(max(16, ((lmax + 3) // 4) * 4))
    return assign, tuple(lpads)


def _prepare_core_inputs(x, per_expert, gup, down, assign, lpads):
    x16 = np.ascontiguousarray(np.asarray(x, dtype=np.float32)).astype(BF16)
    gup16 = np.asarray(gup, dtype=np.float32).astype(BF16)
    down16 = np.asarray(down, dtype=np.float32).astype(BF16)
    xt_sizes = [KD * 128 * lp for lp in lpads]
    xt_off = np.concatenate([[0], np.cumsum(xt_sizes)]).tolist()

    in_maps = []
    for m in range(NCORE):
        xt_buf = np.zeros(xt_off[-1], BF16)
        gup_buf = np.empty((NSLOT, 2, KI, 128, KD * 128), BF16)
        down_buf = np.empty((NSLOT, NDC, 128, KI * 128), BF16)
        for j in range(NSLOT):
            LP = lpads[j]
            e = assign[m, j]
            _, toks, _ = per_expert[e]
            n = len(toks)
            xg = np.zeros((LP, DIM), BF16)
            xg[:n] = x16[toks]
            # [LP, KD, 128] -> k-major [KD, 128(p), LP]
            xt = xg.reshape(LP, KD, 128).transpose(1, 2, 0)
            xt_buf[xt_off[j]: xt_off[j + 1]] = np.ascontiguousarray(xt).ravel()
            for half_gu in (0, 1):
                hm = gup16[e, :, half_gu::2]  # [DIM, INTER] gate or up, deinterleaved
                gup_buf[j, half_gu] = (
                    hm.reshape(KD, 128, KI, 128).transpose(2, 1, 0, 3)
                    .reshape(KI, 128, KD * 128)
                )
            dm = down16[e]  # [INTER, DIM] natural layout, chunked by 128 d-cols
            down_buf[j] = (
                dm.reshape(KI, 128, NDC, 128).transpose(2, 1, 0, 3)
                .reshape(NDC, 128, KI * 128)
            )
        in_maps.append({"xt": xt_buf, "gup": gup_buf, "down": down_buf})
    return in_maps


def _run(inputs: dict, trace: bool = False, tmpdir=None):
    from concourse.bass_utils import run_bass_kernel_spmd

    x = inputs["x"]
    gup = inputs["gate_and_up_projs"]
    down = inputs["down_projs"]

    per_expert = _route(inputs["indices"], inputs["token_mask"], inputs["weights"])
    assign, lpads = _pack_slots(per_expert)

    if lpads not in _PROG_CACHE:
        _PROG_CACHE[lpads] = _build_program(lpads)
    nc, names = _PROG_CACHE[lpads]

    core_maps = _prepare_core_inputs(x, per_expert, gup, down, assign, lpads)
    in_maps = [{names[k]: v for k, v in mm.items()} for mm in core_maps]
    res = run_bass_kernel_spmd(
        nc, in_maps, list(range(NCORE)), trace=trace, tmpdir=tmpdir,
    )

    TOT = sum(lpads)
    toff = np.concatenate([[0], np.cumsum(lpads)]).tolist()
    # yT per core: [NDC, 128, TOT] -> [DIM, TOT]
    Y = np.stack([np.asarray(res.results[m][names["y"]]).reshape(DIM, TOT)
                  for m in range(NCORE)])  # [NCORE, DIM, TOT]

    T = N_TOKENS * TOPK
    core_of = np.zeros(T, np.int64)
    col_of = np.zeros(T, np.int64)
    wgt = np.zeros(T, np.float32)
    slot_of = {int(assign[m, j]): (m, j) for m in range(NCORE) for j in range(NSLOT)}
    for e in range(N_EXPERTS):
        ids, _, ws = per_expert[e]
        m, j = slot_of[e]
        core_of[ids] = m
        col_of[ids] = toff[j] + np.arange(len(ids))
        wgt[ids] = ws / ALPHA          # fold silu(a*g)/a into the combine
    contrib = Y[core_of, :, col_of]    # [T, DIM]
    out = (contrib * wgt[:, None]).reshape(N_TOKENS, TOPK, DIM).sum(axis=1)
    return np.ascontiguousarray(out, dtype=np.float32), res


def kernel(**inputs) -> np.ndarray:
    out, _ = _run(inputs, trace=False)
    return out


# revision 16
# speedup vs baseline: 1.3527x; 1.0032x over previous
"""MoE grouped-experts kernel for Trainium2 (8 NeuronCores, expert-parallel).

Strategy
--------
Expert-parallel: 32 experts packed onto 8 cores x 4 slots. Routing
(sort-by-expert, capacity truncation at the reference's C=1024) is computed
on host from the tiny `indices` tensor; token rows are gathered per expert,
zero-padded to the slot's streamed length, and pre-transposed so the device
kernel is a pure stream of bf16 matmuls (fp32 PSUM accumulation) with zero
on-device transposes:

  GEMM1 (h^T orientation):  hT[m,c] = sum_k gup[k,m] * xT[k,c]
        stationary = gup tile [128 D-rows, 128 cols-of-2I], moving = xT
  act:  aT = silu(1.702*min(gate,7)) * (clip(up,-7,7)+1)   (the 1/1.702 is
        folded into the routing probs applied in the host-side combine)
  GEMM2 (y^T orientation):  yT[d,c] = sum_k down[k,d] * aT[k,c]
        stationary = natural down chunk [128 I-rows, 128 D-cols],
        moving = aT tokens.  Output leaves the device transposed
        [16 d-chunks, 128, tokens]; the host combine untransposes.

Per-slot token streams are trimmed to the max *actual* expert load in the
slot (padded to a multiple of 16), not to 128-row blocks: streamed columns
beyond an expert's load are zeros on host, flow through as zeros, and are
never gathered.  Routing probs (and the 1/1.702 silu fold) are applied on
the host during the combine, so the device emits unscaled yT.  All operands
are bf16 (PSUM accumulates fp32), halving HBM traffic and SBUF pressure
versus fp32.

DMA pacing: the sync ring is a single FIFO fanned over 16 engines, so
every transfer is issued in need-order and sized so nothing blocks a
latency-critical load: xT is split per contraction tile (139 KB), gup
stationaries are issued two (i,half) steps ahead across slot boundaries,
and the next slot's xT is paced between GEMM2's down-chunk loads.
"""

import math
from contextlib import ExitStack

import numpy as np
import ml_dtypes

BF16 = ml_dtypes.bfloat16

N_TOKENS, DIM = 4096, 2048
N_EXPERTS, TOPK, INTER = 32, 4, 1408
ALPHA, LIMIT, LIN_OFFSET = 1.702, 7.0, 1.0

NCORE = 8
NSLOT = N_EXPERTS // NCORE        # expert slots per core = 4
KD = DIM // 128                   # 16 contraction tiles for GEMM1
KI = INTER // 128                 # 11 contraction tiles for GEMM2
NDC = DIM // 128                  # 16 output d-chunks for GEMM2
C_REF = 2 * ((N_TOKENS * TOPK + N_EXPERTS - 1) // N_EXPERTS)  # 1024

_PROG_CACHE: dict = {}


def _groups(lpad: int):
    """Split lpad into PSUM-bank-sized moving groups (<=512, mult of 2)."""
    ng = max(1, math.ceil(lpad / 512))
    per = (lpad // ng // 2) * 2
    sizes = [per] * (ng - 1) + [lpad - per * (ng - 1)]
    assert all(0 < s <= 512 for s in sizes), sizes
    out, off = [], 0
    for s in sizes:
        out.append((off, s))
        off += s
    return out


def _build_program(lpads: tuple):
    import concourse.bacc as bacc
    import concourse.mybir as mybir
    import concourse.tile as tile
    from concourse.alu_op_type import AluOpType

    F32 = mybir.dt.float32
    BF = mybir.dt.bfloat16
    TOT = sum(lpads)
    lmax = max(lpads)
    toff = np.concatenate([[0], np.cumsum(lpads)]).tolist()
    xt_sizes = [KD * 128 * lp for lp in lpads]
    xt_off = np.concatenate([[0], np.cumsum(xt_sizes)]).tolist()

    nc = bacc.Bacc(None, target_bir_lowering=False, debug=False)
    with ExitStack() as ctx:
        tc = ctx.enter_context(tile.TileContext(nc))
        dram = ctx.enter_context(tc.tile_pool(name="dram", bufs=1, space="DRAM"))
        xt_d = dram.tile([xt_off[-1]], BF, kind="ExternalInput")
        gup_d = dram.tile([NSLOT, 2, KI, 128, KD * 128], BF, kind="ExternalInput")
        down_d = dram.tile([NSLOT, NDC, 128, KI * 128], BF, kind="ExternalInput")
        y_d = dram.tile([NDC, 128, TOT], F32, kind="ExternalOutput")
        names = {"xt": xt_d.name, "gup": gup_d.name, "down": down_d.name,
                 "y": y_d.name}

        xt_pool = ctx.enter_context(tc.tile_pool(name="xt", bufs=2 * KD + 2))
        gup_pool = ctx.enter_context(tc.tile_pool(name="gup", bufs=6))
        gup0_pool = ctx.enter_context(tc.tile_pool(name="gup0", bufs=4))
        down_pool = ctx.enter_context(tc.tile_pool(name="down", bufs=4))
        at_pool = ctx.enter_context(tc.tile_pool(name="at", bufs=2))
        fg_pool = ctx.enter_context(tc.tile_pool(name="fg", bufs=3))
        tmp_pool = ctx.enter_context(tc.tile_pool(name="tmp", bufs=4))
        y_pool = ctx.enter_context(tc.tile_pool(name="yt", bufs=4))
        psg = ctx.enter_context(tc.tile_pool(name="psg", bufs=8, space="PSUM"))

        def load_xt_k(j, k):
            """One contraction tile [128, lp] of slot j's xT."""
            lp = lpads[j]
            t = xt_pool.tile([128, lmax], BF, tag="xt")
            src = xt_d[xt_off[j] + k * 128 * lp: xt_off[j] + (k + 1) * 128 * lp]
            nc.sync.dma_start(out=t[:, :lp],
                              in_=src.rearrange("(p c) -> p c", p=128))
            return t

        # flat (j, i, half) order of gup stationary loads, prefetched depth-2
        gsteps = [(j, i, half)
                  for j in range(NSLOT) for i in range(KI) for half in (0, 1)]
        gup_tiles: dict = {}

        def issue_gup(s):
            if s < len(gsteps):
                j, i, half = gsteps[s]
                t = gup_pool.tile([128, KD * 128], BF, tag="gup")
                nc.sync.dma_start(out=t[:], in_=gup_d[j, half, i])
                gup_tiles[s] = t

        # first two stationaries are split in k-halves, interleaved with the
        # xT tiles in consumption order, so matmul k=0 starts early and the
        # first two sweeps ride just behind the DMA ring
        def issue_gup_split(s):
            j, i, half = gsteps[s]
            hw_ = (KD // 2) * 128
            ha = gup0_pool.tile([128, hw_], BF, tag="gup0")
            nc.sync.dma_start(out=ha[:], in_=gup_d[j, half, i, :, :hw_])
            return ha

        def issue_gup_split2(s, ha):
            j, i, half = gsteps[s]
            hw_ = (KD // 2) * 128
            hb = gup0_pool.tile([128, hw_], BF, tag="gup0")
            nc.sync.dma_start(out=hb[:], in_=gup_d[j, half, i, :, hw_:])
            gup_tiles[s] = (ha, hb)

        _h0 = issue_gup_split(0)
        xt_tiles = [load_xt_k(0, 0)]
        issue_gup_split2(0, _h0)
        xt_tiles += [load_xt_k(0, k) for k in range(1, 8)]
        _h1 = issue_gup_split(1)
        xt_tiles += [load_xt_k(0, k) for k in range(8, KD)]
        issue_gup_split2(1, _h1)

        down_tiles: dict = {}

        def issue_down(j, dc):
            t = down_pool.tile([128, KI * 128], BF, tag="down")
            nc.sync.dma_start(out=t[:], in_=down_d[j, dc])
            down_tiles[dc] = t

        for j in range(NSLOT):
            LP = lpads[j]
            groups = _groups(LP)
            xt_h = xt_tiles
            next_xt: list = []

            at_sb = at_pool.tile([128, KI * lmax], BF, tag="at")

            for i in range(KI):
                for half in (0, 1):  # 0 = gate, 1 = up
                    s = (j * KI + i) * 2 + half
                    issue_gup(s + 2)
                    if half == 1 and j + 1 < NSLOT and 3 <= i <= KI - 1:
                        # pace next slot's xT through GEMM1's ring slack
                        for k in (2 * (i - 3), 2 * (i - 3) + 1):
                            if k < KD:
                                next_xt.append(load_xt_k(j + 1, k))
                    if i == KI - 2 and half == 1:
                        issue_down(j, 0)
                        issue_down(j, 1)
                    gsb = gup_tiles.pop(s)
                    if isinstance(gsb, tuple):
                        def g_ap(k, gsb=gsb):
                            h = KD // 2
                            return gsb[k // h][:, (k % h) * 128:(k % h + 1) * 128]
                    else:
                        def g_ap(k, gsb=gsb):
                            return gsb[:, k * 128:(k + 1) * 128]
                    pss = [psg.tile([128, 512], F32, tag="ps",
                                    name=f"ps1_{j}_{i}_{half}_{gi}")
                           for gi in range(len(groups))]
                    for k in range(KD):
                        for gi, (g0, gw) in enumerate(groups):
                            nc.tensor.matmul(
                                pss[gi][:, :gw],
                                lhsT=g_ap(k),
                                rhs=xt_h[k][:, g0:g0 + gw],
                                start=(k == 0), stop=(k == KD - 1),
                            )
                    for gi, (g0, gw) in enumerate(groups):
                        ps = pss[gi]
                        if half == 0:
                            t0 = tmp_pool.tile([128, 512], F32, tag="t0")
                            nc.vector.tensor_scalar_min(t0[:, :gw], ps[:, :gw], LIMIT)
                            fg = fg_pool.tile([128, 512], F32, tag="fg")
                            nc.scalar.activation(
                                fg[:, :gw], t0[:, :gw],
                                mybir.ActivationFunctionType.Silu, scale=ALPHA,
                            )
                            if gi == 0:
                                fgs = [fg]
                            else:
                                fgs.append(fg)
                        else:
                            uc = tmp_pool.tile([128, 512], F32, tag="uc")
                            nc.vector.tensor_scalar(
                                uc[:, :gw], ps[:, :gw], LIMIT, -LIMIT,
                                AluOpType.min, AluOpType.max,
                            )
                            # aT = (clip(up)+1) * silu(1.702*min(gate,7))
                            nc.vector.scalar_tensor_tensor(
                                at_sb[:, i * LP + g0: i * LP + g0 + gw],
                                uc[:, :gw], LIN_OFFSET, fgs[gi][:, :gw],
                                AluOpType.add, AluOpType.mult,
                            )

            for dc in range(NDC):
                if dc + 2 < NDC:
                    issue_down(j, dc + 2)
                dsb = down_tiles.pop(dc)
                ps2s = [psg.tile([128, 512], F32, tag="ps",
                                 name=f"ps2_{j}_{dc}_{gi}")
                        for gi in range(len(groups))]
                for k in range(KI):
                    for gi, (g0, gw) in enumerate(groups):
                        nc.tensor.matmul(
                            ps2s[gi][:, :gw],
                            lhsT=dsb[:, k * 128:(k + 1) * 128],
                            rhs=at_sb[:, k * LP + g0: k * LP + g0 + gw],
                            start=(k == 0), stop=(k == KI - 1),
                        )
                # split the very last evictions so the tail drains quickly
                nev = 2 if (j == NSLOT - 1 and dc == NDC - 1) else 1
                for gi, (g0, gw) in enumerate(groups):
                    cw = (gw + nev - 1) // nev
                    for v in range(nev):
                        a, b = v * cw, min((v + 1) * cw, gw)
                        yt = y_pool.tile([128, 512], F32, tag="yt")
                        nc.scalar.activation(
                            yt[:, :b - a], ps2s[gi][:, a:b],
                            mybir.ActivationFunctionType.Copy,
                        )
                        nc.sync.dma_start(
                            out=y_d[dc, :, toff[j] + g0 + a: toff[j] + g0 + b],
                            in_=yt[:, :b - a],
                        )
            xt_tiles = next_xt
    nc.compile()
    return nc, names


def _route(indices, token_mask, weights):
    """Replicate the reference's permute/capacity semantics on host."""
    idx = np.asarray(indices).astype(np.int64)
    mask = np.asarray(token_mask).astype(bool)
    w = np.asarray(weights).astype(np.float32)
    flat_e = np.where(mask[:, None], idx, -1).ravel()
    w_flat = np.where(flat_e >= 0, w.ravel(), 0.0).astype(np.float32)
    tok = np.repeat(np.arange(N_TOKENS, dtype=np.int64), TOPK)

    per_expert = []  # (first flat_ids, unique token_ids, merged weights)
    for e in range(N_EXPERTS):
        ids = np.nonzero(flat_e == e)[0][:C_REF]
        # a token routed to the same expert k times contributes (w1+..+wk)*y;
        # merge duplicates so each (token, expert) pair is computed once
        ut, first_idx, inv = np.unique(tok[ids], return_index=True,
                                       return_inverse=True)
        uw = np.bincount(inv, weights=w_flat[ids]).astype(np.float32)
        per_expert.append((ids[first_idx], ut, uw))
    return per_expert


def _pack_slots(per_expert):
    """Assign experts to (core, slot); slot stream length = max load in slot."""
    loads = [len(t) for _, t, _ in per_expert]
    order = sorted(range(N_EXPERTS), key=lambda e: -loads[e])
    assign = np.empty((NCORE, NSLOT), np.int64)
    lpads = []
    for j in range(NSLOT):
        col = order[j * NCORE:(j + 1) * NCORE]
        for m in range(NCORE):
            assign[m, j] = col[m]
        lmax = max(loads[e] for e in col)
        lpads.append(max(16, ((lmax + 3) // 4) * 4))
    return assign, tuple(lpads)


def _prepare_core_inputs(x, per_expert, gup, down, assign, lpads):
    x16 = np.ascontiguousarray(np.asarray(x, dtype=np.float32)).astype(BF16)
    gup16 = np.asarray(gup, dtype=np.float32).astype(BF16)
    down16 = np.asarray(down, dtype=np.float32).astype(BF16)
    xt_sizes = [KD * 128 * lp for lp in lpads]
    xt_off = np.concatenate([[0], np.cumsum(xt_sizes)]).tolist()

    in_maps = []
    for m in range(NCORE):
        xt_buf = np.zeros(xt_off[-1], BF16)
        gup_buf = np.empty((NSLOT, 2, KI, 128, KD * 128), BF16)
        down_buf = np.empty((NSLOT, NDC, 128, KI * 128), BF16)
        for j in range(NSLOT):
            LP = lpads[j]
            e = assign[m, j]
            _, toks, _ = per_expert[e]
            n = len(toks)
            xg = np.zeros((LP, DIM), BF16)
            xg[:n] = x16[toks]
            # [LP, KD, 128] -> k-major [KD, 128(p), LP]
            xt = xg.reshape(LP, KD, 128).transpose(1, 2, 0)
            xt_buf[xt_off[j]: xt_off[j + 1]] = np.ascontiguousarray(xt).ravel()
            for half_gu in (0, 1):
                hm = gup16[e, :, half_gu::2]  # [DIM, INTER] gate or up, deinterleaved
                gup_buf[j, half_gu] = (
                    hm.reshape(KD, 128, KI, 128).transpose(2, 1, 0, 3)
                    .reshape(KI, 128, KD * 128)
                )
            dm = down16[e]  # [INTER, DIM] natural layout, chunked by 128 d-cols
            down_buf[j] = (
                dm.reshape(KI, 128, NDC, 128).transpose(2, 1, 0, 3)
                .reshape(NDC, 128, KI * 128)
            )
        in_maps.append({"xt": xt_buf, "gup": gup_buf, "down": down_buf})
    return in_maps


def _run(inputs: dict, trace: bool = False, tmpdir=None):
    from concourse.bass_utils import run_bass_kernel_spmd

    x = inputs["x"]
    gup = inputs["gate_and_up_projs"]
    down = inputs["down_projs"]

    per_expert = _route(inputs["indices"], inputs["token_mask"], inputs["weights"])
    assign, lpads = _pack_slots(per_expert)

    if lpads not in _PROG_CACHE:
        _PROG_CACHE[lpads] = _build_program(lpads)
    nc, names = _PROG_CACHE[lpads]

    core_maps = _prepare_core_inputs(x, per_expert, gup, down, assign, lpads)
    in_maps = [{names[k]: v for k, v in mm.items()} for mm in core_maps]
    res = run_bass_kernel_spmd(
        nc, in_maps, list(range(NCORE)), trace=trace, tmpdir=tmpdir,
    )

    TOT = sum(lpads)
    toff = np.concatenate([[0], np.cumsum(lpads)]).tolist()
    # yT per core: [NDC, 128, TOT] -> [DIM, TOT]
    Y = np.stack([np.asarray(res.results[m][names["y"]]).reshape(DIM, TOT)
                  for m in range(NCORE)])  # [NCORE, DIM, TOT]

    T = N_TOKENS * TOPK
    core_of = np.zeros(T, np.int64)
    col_of = np.zeros(T, np.int64)
    wgt = np.zeros(T, np.float32)
    slot_of = {int(assign[m, j]): (m, j) for m in range(NCORE) for j in range(NSLOT)}
    for e in range(N_EXPERTS):
        ids, _, ws = per_expert[e]
        m, j = slot_of[e]
        core_of[ids] = m
        col_of[ids] = toff[j] + np.arange(len(ids))
        wgt[ids] = ws / ALPHA          # fold silu(a*g)/a into the combine
    contrib = Y[core_of, :, col_of]    # [T, DIM]
    out = (contrib * wgt[:, None]).reshape(N_TOKENS, TOPK, DIM).sum(axis=1)
    return np.ascontiguousarray(out, dtype=np.float32), res


def kernel(**inputs) -> np.ndarray:
    out, _ = _run(inputs, trace=False)
    return out


# revision 31
# speedup vs baseline: 1.3703x; 1.0130x over previous
"""MoE grouped-experts kernel for Trainium2 (8 NeuronCores, expert-parallel).

Strategy
--------
Expert-parallel: 32 experts packed onto 8 cores x 4 slots. Routing
(sort-by-expert, capacity truncation at the reference's C=1024) is computed
on host from the tiny `indices` tensor; token rows are gathered per expert,
zero-padded to the slot's streamed length, and pre-transposed so the device
kernel is a pure stream of bf16 matmuls (fp32 PSUM accumulation) with zero
on-device transposes:

  GEMM1 (h^T orientation):  hT[m,c] = sum_k gup[k,m] * xT[k,c]
        stationary = gup tile [128 D-rows, 128 cols-of-2I], moving = xT
  act:  aT = silu(1.702*min(gate,7)) * (clip(up,-7,7)+1)   (the 1/1.702 is
        folded into the routing probs applied in the host-side combine)
  GEMM2 (y^T orientation):  yT[d,c] = sum_k down[k,d] * aT[k,c]
        stationary = natural down chunk [128 I-rows, 128 D-cols],
        moving = aT tokens.  Output leaves the device transposed
        [16 d-chunks, 128, tokens]; the host combine untransposes.

Per-slot token streams are trimmed to the max *actual* expert load in the
slot (padded to a multiple of 16), not to 128-row blocks: streamed columns
beyond an expert's load are zeros on host, flow through as zeros, and are
never gathered.  Routing probs (and the 1/1.702 silu fold) are applied on
the host during the combine, so the device emits unscaled yT.  All operands
are bf16 (PSUM accumulates fp32), halving HBM traffic and SBUF pressure
versus fp32.

DMA pacing: the sync ring is a single FIFO fanned over 16 engines, so
every transfer is issued in need-order and sized so nothing blocks a
latency-critical load: xT is split per contraction tile (139 KB), gup
stationaries are issued two (i,half) steps ahead across slot boundaries,
and the next slot's xT is paced between GEMM2's down-chunk loads.
"""

import math
from contextlib import ExitStack

import numpy as np
import ml_dtypes

BF16 = ml_dtypes.bfloat16

N_TOKENS, DIM = 4096, 2048
N_EXPERTS, TOPK, INTER = 32, 4, 1408
ALPHA, LIMIT, LIN_OFFSET = 1.702, 7.0, 1.0

NCORE = 8
NSLOT = N_EXPERTS // NCORE        # expert slots per core = 4
KD = DIM // 128                   # 16 contraction tiles for GEMM1
KI = INTER // 128                 # 11 contraction tiles for GEMM2
NDC = DIM // 128                  # 16 output d-chunks for GEMM2
C_REF = 2 * ((N_TOKENS * TOPK + N_EXPERTS - 1) // N_EXPERTS)  # 1024
XT_NSLAB = (4,) + (2,) * (NSLOT - 1)  # xT DMA slabs per slot

_PROG_CACHE: dict = {}


def _groups(lpad: int):
    """Split lpad into PSUM-bank-sized moving groups (<=512, mult of 2)."""
    ng = max(1, math.ceil(lpad / 512))
    per = (lpad // ng // 2) * 2
    sizes = [per] * (ng - 1) + [lpad - per * (ng - 1)]
    assert all(0 < s <= 512 for s in sizes), sizes
    out, off = [], 0
    for s in sizes:
        out.append((off, s))
        off += s
    return out


def _build_program(lpads: tuple):
    import concourse.bacc as bacc
    import concourse.mybir as mybir
    import concourse.tile as tile
    from concourse.alu_op_type import AluOpType

    F32 = mybir.dt.float32
    BF = mybir.dt.bfloat16
    TOT = sum(lpads)
    lmax = max(lpads)
    toff = np.concatenate([[0], np.cumsum(lpads)]).tolist()
    xt_sizes = [KD * 128 * lp for lp in lpads]
    xt_off = np.concatenate([[0], np.cumsum(xt_sizes)]).tolist()
    # slot 0 streams in 4 quarter-slabs (fast first matmul); rest in halves
    xt_nslab = XT_NSLAB

    nc = bacc.Bacc(None, target_bir_lowering=False, debug=False)
    with ExitStack() as ctx:
        tc = ctx.enter_context(tile.TileContext(nc))
        dram = ctx.enter_context(tc.tile_pool(name="dram", bufs=1, space="DRAM"))
        xt_d = dram.tile([xt_off[-1]], BF, kind="ExternalInput")
        gup_d = dram.tile([NSLOT, 2, KI, 128, KD * 128], BF, kind="ExternalInput")
        down_d = dram.tile([NSLOT, NDC // 2, 128, 2 * KI * 128], BF,
                           kind="ExternalInput")
        y_d = dram.tile([NDC, 128, TOT], F32, kind="ExternalOutput")
        names = {"xt": xt_d.name, "gup": gup_d.name, "down": down_d.name,
                 "y": y_d.name}

        xtq_pool = ctx.enter_context(tc.tile_pool(name="xtq", bufs=4))
        xth_pool = ctx.enter_context(tc.tile_pool(name="xth", bufs=4))
        gup_pool = ctx.enter_context(tc.tile_pool(name="gup", bufs=6))
        gup0_pool = ctx.enter_context(tc.tile_pool(name="gup0", bufs=4))
        down_pool = ctx.enter_context(tc.tile_pool(name="down", bufs=4))
        at_pool = ctx.enter_context(tc.tile_pool(name="at", bufs=2))
        fg_pool = ctx.enter_context(tc.tile_pool(name="fg", bufs=3))
        tmp_pool = ctx.enter_context(tc.tile_pool(name="tmp", bufs=4))
        y_pool = ctx.enter_context(tc.tile_pool(name="yt", bufs=4))
        psg = ctx.enter_context(tc.tile_pool(name="psg", bufs=8, space="PSUM"))

        def load_xt_slab(j, si):
            """One contiguous p-major slab (KD/nslab k-tiles) of slot j's xT."""
            lp = lpads[j]
            nk = KD // xt_nslab[j]
            pool, tag = (xtq_pool, "xtq") if xt_nslab[j] == 4 else (xth_pool, "xth")
            t = pool.tile([128, nk * lmax], BF, tag=tag)
            blk = 128 * nk * lp
            src = xt_d[xt_off[j] + si * blk: xt_off[j] + (si + 1) * blk]
            nc.sync.dma_start(out=t[:, :nk * lp],
                              in_=src.rearrange("(p c) -> p c", p=128))
            return t

        # flat (j, i, half) order of gup stationary loads, prefetched depth-2
        gsteps = [(j, i, half)
                  for j in range(NSLOT) for i in range(KI) for half in (0, 1)]
        gup_tiles: dict = {}

        def issue_gup(s):
            if s < len(gsteps):
                j, i, half = gsteps[s]
                t = gup_pool.tile([128, KD * 128], BF, tag="gup")
                nc.sync.dma_start(out=t[:], in_=gup_d[j, half, i])
                gup_tiles[s] = t

        # first two stationaries are split in k-halves, interleaved with the
        # xT tiles in consumption order, so matmul k=0 starts early and the
        # first two sweeps ride just behind the DMA ring
        def issue_gup_split(s):
            j, i, half = gsteps[s]
            hw_ = (KD // 2) * 128
            ha = gup0_pool.tile([128, hw_], BF, tag="gup0")
            nc.sync.dma_start(out=ha[:], in_=gup_d[j, half, i, :, :hw_])
            return ha

        def issue_gup_split2(s, ha):
            j, i, half = gsteps[s]
            hw_ = (KD // 2) * 128
            hb = gup0_pool.tile([128, hw_], BF, tag="gup0")
            nc.sync.dma_start(out=hb[:], in_=gup_d[j, half, i, :, hw_:])
            gup_tiles[s] = (ha, hb)

        _h0 = issue_gup_split(0)
        xt_tiles = [load_xt_slab(0, 0)]
        issue_gup_split2(0, _h0)
        xt_tiles.append(load_xt_slab(0, 1))
        _h1 = issue_gup_split(1)
        xt_tiles.append(load_xt_slab(0, 2))
        issue_gup_split2(1, _h1)
        xt_tiles.append(load_xt_slab(0, 3))

        down_tiles: dict = {}

        def issue_down(j, pd):
            """Load pair pd = down chunks (2pd, 2pd+1) in one transfer."""
            t = down_pool.tile([128, 2 * KI * 128], BF, tag="down")
            nc.sync.dma_start(out=t[:], in_=down_d[j, pd])
            down_tiles[pd] = t

        for j in range(NSLOT):
            LP = lpads[j]
            groups = _groups(LP)
            xt_h = xt_tiles
            next_xt: list = []
            nkj = KD // xt_nslab[j]

            def xt_ap(k, g0, gw, LP=LP, xt_h=xt_h, nkj=nkj):
                off = (k % nkj) * LP
                return xt_h[k // nkj][:, off + g0: off + g0 + gw]

            at_sb = at_pool.tile([128, KI * lmax], BF, tag="at")

            for i in range(KI):
                for half in (0, 1):  # 0 = gate, 1 = up
                    s = (j * KI + i) * 2 + half
                    issue_gup(s + 2)
                    if half == 1 and j + 1 < NSLOT and i in (4, 7):
                        # pace next slot's xT through GEMM1's ring slack
                        next_xt.append(load_xt_slab(j + 1, len(next_xt)))
                    if i == KI - 2 and half == 1:
                        issue_down(j, 0)
                    gsb = gup_tiles.pop(s)
                    if isinstance(gsb, tuple):
                        def g_ap(k, gsb=gsb):
                            h = KD // 2
                            return gsb[k // h][:, (k % h) * 128:(k % h + 1) * 128]
                    else:
                        def g_ap(k, gsb=gsb):
                            return gsb[:, k * 128:(k + 1) * 128]
                    pss = [psg.tile([128, 512], F32, tag="ps",
                                    name=f"ps1_{j}_{i}_{half}_{gi}")
                           for gi in range(len(groups))]
                    for k in range(KD):
                        for gi, (g0, gw) in enumerate(groups):
                            nc.tensor.matmul(
                                pss[gi][:, :gw],
                                lhsT=g_ap(k),
                                rhs=xt_ap(k, g0, gw),
                                start=(k == 0), stop=(k == KD - 1),
                            )
                    for gi, (g0, gw) in enumerate(groups):
                        ps = pss[gi]
                        if half == 0:
                            t0 = tmp_pool.tile([128, 512], F32, tag="t0")
                            nc.vector.tensor_scalar_min(t0[:, :gw], ps[:, :gw], LIMIT)
                            fg = fg_pool.tile([128, 512], F32, tag="fg")
                            nc.scalar.activation(
                                fg[:, :gw], t0[:, :gw],
                                mybir.ActivationFunctionType.Silu, scale=ALPHA,
                            )
                            if gi == 0:
                                fgs = [fg]
                            else:
                                fgs.append(fg)
                        else:
                            uc = tmp_pool.tile([128, 512], F32, tag="uc")
                            nc.vector.tensor_scalar(
                                uc[:, :gw], ps[:, :gw], LIMIT, -LIMIT,
                                AluOpType.min, AluOpType.max,
                            )
                            # aT = (clip(up)+1) * silu(1.702*min(gate,7))
                            nc.vector.scalar_tensor_tensor(
                                at_sb[:, i * LP + g0: i * LP + g0 + gw],
                                uc[:, :gw], LIN_OFFSET, fgs[gi][:, :gw],
                                AluOpType.add, AluOpType.mult,
                            )

            for dc in range(NDC):
                if dc % 2 == 0 and dc + 2 < NDC:
                    issue_down(j, (dc + 2) // 2)
                dsb = down_tiles[dc // 2]
                sub = (dc % 2) * KI
                ps2s = [psg.tile([128, 512], F32, tag="ps",
                                 name=f"ps2_{j}_{dc}_{gi}")
                        for gi in range(len(groups))]
                for k in range(KI):
                    for gi, (g0, gw) in enumerate(groups):
                        nc.tensor.matmul(
                            ps2s[gi][:, :gw],
                            lhsT=dsb[:, (sub + k) * 128:(sub + k + 1) * 128],
                            rhs=at_sb[:, k * LP + g0: k * LP + g0 + gw],
                            start=(k == 0), stop=(k == KI - 1),
                        )
                # evict the full dc row as one write on the idle DVE queue
                yt = y_pool.tile([128, lmax], F32, tag="yt")
                for gi, (g0, gw) in enumerate(groups):
                    nc.scalar.activation(
                        yt[:, g0:g0 + gw], ps2s[gi][:, :gw],
                        mybir.ActivationFunctionType.Copy,
                    )
                nc.scalar.dma_start(
                    out=y_d[dc, :, toff[j]: toff[j] + LP],
                    in_=yt[:, :LP],
                )
            xt_tiles = next_xt
    nc.compile()
    return nc, names


def _route(indices, token_mask, weights):
    """Replicate the reference's permute/capacity semantics on host."""
    idx = np.asarray(indices).astype(np.int64)
    mask = np.asarray(token_mask).astype(bool)
    w = np.asarray(weights).astype(np.float32)
    flat_e = np.where(mask[:, None], idx, -1).ravel()
    w_flat = np.where(flat_e >= 0, w.ravel(), 0.0).astype(np.float32)
    tok = np.repeat(np.arange(N_TOKENS, dtype=np.int64), TOPK)

    per_expert = []  # (first flat_ids, unique token_ids, merged weights)
    for e in range(N_EXPERTS):
        ids = np.nonzero(flat_e == e)[0][:C_REF]
        # a token routed to the same expert k times contributes (w1+..+wk)*y;
        # merge duplicates so each (token, expert) pair is computed once
        ut, first_idx, inv = np.unique(tok[ids], return_index=True,
                                       return_inverse=True)
        uw = np.bincount(inv, weights=w_flat[ids]).astype(np.float32)
        per_expert.append((ids[first_idx], ut, uw))
    return per_expert


def _pack_slots(per_expert):
    """Assign experts to (core, slot); slot stream length = max load in slot."""
    loads = [len(t) for _, t, _ in per_expert]
    order = sorted(range(N_EXPERTS), key=lambda e: -loads[e])
    assign = np.empty((NCORE, NSLOT), np.int64)
    lpads = []
    for j in range(NSLOT):
        col = order[j * NCORE:(j + 1) * NCORE]
        for m in range(NCORE):
            assign[m, j] = col[m]
        lmax = max(loads[e] for e in col)
        lpads.append(max(16, ((lmax + 3) // 4) * 4))
    return assign, tuple(lpads)


def _prepare_core_inputs(x, per_expert, gup, down, assign, lpads):
    x16 = np.ascontiguousarray(np.asarray(x, dtype=np.float32)).astype(BF16)
    gup16 = np.asarray(gup, dtype=np.float32).astype(BF16)
    down16 = np.asarray(down, dtype=np.float32).astype(BF16)
    xt_sizes = [KD * 128 * lp for lp in lpads]
    xt_off = np.concatenate([[0], np.cumsum(xt_sizes)]).tolist()

    in_maps = []
    for m in range(NCORE):
        xt_buf = np.zeros(xt_off[-1], BF16)
        gup_buf = np.empty((NSLOT, 2, KI, 128, KD * 128), BF16)
        down_buf = np.empty((NSLOT, NDC // 2, 128, 2 * KI * 128), BF16)
        for j in range(NSLOT):
            LP = lpads[j]
            e = assign[m, j]
            _, toks, _ = per_expert[e]
            n = len(toks)
            xg = np.zeros((LP, DIM), BF16)
            xg[:n] = x16[toks]
            # p-major slabs: each [128(p), nk, LP] contiguous
            xt = xg.reshape(LP, KD, 128).transpose(2, 1, 0)  # [128, KD, LP]
            nk = KD // XT_NSLAB[j]
            xt_buf[xt_off[j]: xt_off[j + 1]] = np.concatenate(
                [np.ascontiguousarray(xt[:, si * nk:(si + 1) * nk, :]).ravel()
                 for si in range(XT_NSLAB[j])])
            for half_gu in (0, 1):
                hm = gup16[e, :, half_gu::2]  # [DIM, INTER] gate or up, deinterleaved
                gup_buf[j, half_gu] = (
                    hm.reshape(KD, 128, KI, 128).transpose(2, 1, 0, 3)
                    .reshape(KI, 128, KD * 128)
                )
            dm = down16[e]  # [INTER, DIM] natural layout, chunked by 128 d-cols
            down_buf[j] = (
                dm.reshape(KI, 128, NDC, 128).transpose(2, 1, 0, 3)
                .reshape(NDC // 2, 2, 128, KI * 128).swapaxes(1, 2)
                .reshape(NDC // 2, 128, 2 * KI * 128)
            )
        in_maps.append({"xt": xt_buf, "gup": gup_buf, "down": down_buf})
    return in_maps


def _run(inputs: dict, trace: bool = False, tmpdir=None):
    from concourse.bass_utils import run_bass_kernel_spmd

    x = inputs["x"]
    gup = inputs["gate_and_up_projs"]
    down = inputs["down_projs"]

    per_expert = _route(inputs["indices"], inputs["token_mask"], inputs["weights"])
    assign, lpads = _pack_slots(per_expert)

    if lpads not in _PROG_CACHE:
        _PROG_CACHE[lpads] = _build_program(lpads)
    nc, names = _PROG_CACHE[lpads]

    core_maps = _prepare_core_inputs(x, per_expert, gup, down, assign, lpads)
    in_maps = [{names[k]: v for k, v in mm.items()} for mm in core_maps]
    res = run_bass_kernel_spmd(
        nc, in_maps, list(range(NCORE)), trace=trace, tmpdir=tmpdir,
    )

    TOT = sum(lpads)
    toff = np.concatenate([[0], np.cumsum(lpads)]).tolist()
    # yT per core: [NDC, 128, TOT] -> [DIM, TOT]
    Y = np.stack([np.asarray(res.results[m][names["y"]]).reshape(DIM, TOT)
                  for m in range(NCORE)])  # [NCORE, DIM, TOT]

    T = N_TOKENS * TOPK
    core_of = np.zeros(T, np.int64)
    col_of = np.zeros(T, np.int64)
    wgt = np.zeros(T, np.float32)
    slot_of = {int(assign[m, j]): (m, j) for m in range(NCORE) for j in range(NSLOT)}
    for e in range(N_EXPERTS):
        ids, _, ws = per_expert[e]
        m, j = slot_of[e]
        core_of[ids] = m
        col_of[ids] = toff[j] + np.arange(len(ids))
        wgt[ids] = ws / ALPHA          # fold silu(a*g)/a into the combine
    contrib = Y[core_of, :, col_of]    # [T, DIM]
    out = (contrib * wgt[:, None]).reshape(N_TOKENS, TOPK, DIM).sum(axis=1)
    return np.ascontiguousarray(out, dtype=np.float32), res


def kernel(**inputs) -> np.ndarray:
    out, _ = _run(inputs, trace=False)
    return out
